# revision 43
# baseline (speedup 1.0000x reference)
"""AttnBlock (GroupNorm -> 1x1-conv QKV self-attention -> 1x1-conv out -> residual)
for Trainium2, data-parallel over batch across 8 NeuronCores.

Contract: kernel(**inputs) takes the FULL inputs (np arrays, dtypes as in
setup_inputs) and returns the FULL output [32, 256, 32, 32] fp32.

Math (per batch, all folds exact in real arithmetic, done in fp64 on host):
  h = GroupNorm(x)                                  [C, N]
  scores s[n,m] = (q_n + bq) . (k_m + bk) / 16  with q = wq h, k = wk h
    = (h_n^T M h_m + gam . h_n + w2 . h_m + c2) / 16,
      M = wq^T wk, gam = wq^T bk, w2 = wk^T bq, c2 = bq.bk
  softmax over m; o = attn @ v; out = x + wo o + bo
    wo folded: v' = (wo wv) h, out = x + (v' P^T) + (wo bv + bo)
  Softmax denominators come free from a ones-column appended to the v'
  tiles; no max-subtraction is needed (|s| <= ~9 here, exp is fp32-safe).

Channel layout: c = 2p + j (partition p, slot j in {0,1}) so every x/out DMA
is fully contiguous per partition and each partition's channels belong to a
single group (group g = p // 4, 32 groups -> one stat-reduce mm per batch).
Weight matrices are column-permuted on the host so that every matmul operand
slice on the device is contiguous.

Device dataflow per batch (PIPE=4 pipelined calls x 1 batch per core each):
  xf = xq * xscale (fp16)      [int8 x dequant, per-channel scale]
  g = M^T h + gam (fp32r)      [matmul, ACT Identity w/ bias]
  E[m,n] = exp(s^T) (bf16)     [lhsT=g block, rhs=h chunk; ACT Exp with
                                per-partition bias r2t = (w2.h_m + c2)/16,
                                r2 computed as an extra column of the v' mm]
  v't[m, 0:256] = v' (bf16), [:,256:258] = 1
  U[n, 0:258] = sum_m E[m,nb] v't[m]   (psum);  oT = U[:, :256] / U[:,256]
  oq = int8(oT^T * 127/absmax) [per-channel quant; scales shipped bitcast
                                in the output tensor's tail]

Host<->device transport (the wall-clock bottleneck — the axon tunnel is
driven by a single host vCPU at ~48 MB/s aggregate, effectively
half-duplex, ~73 ms per RPC): x is shipped as per-channel int8 + f32
scales (quant error feeds only the attention path and averages out over
the 256-channel contractions); o + bop comes back as per-channel int8
with its scales bitcast into a tail region of the same flat tensor (one
pull per stage); the residual out = x + oq*scale runs on the host at full
fp32 precision from the exact fp32 x, inline (host CPU time competes
directly with tunnel streaming, so fewer passes beat more threads). The
work is split into PIPE=4 pipeline stages (1 batch/core each) to hide the
per-stage RPC latencies, with copy_to_host_async prefetch so each result
streams as soon as its exec finishes. All folded weights ride in one
packed f32 tensor cached on device across calls (content-hashed), the
jitted executor is built once and reused (a fresh jax.jit per call would
retrace + take the slow numpy-arg transfer path), and no zero output
buffers are uploaded (the kernel writes every output element, so the
custom call result stays uninitialized).
"""
import numpy as np

import concourse.bacc as bacc
import concourse.mybir as mybir
import concourse.tile as tile
from concourse import bass_isa

N_CORES = 8
B, C, H, W = 32, 256, 32, 32
NSP = H * W            # 1024 spatial positions
BL = B // N_CORES      # 4 batches per core
PIPE = 4               # pipeline stages per call (overlap up/down streams)
BLC = BL // PIPE       # batches per core per stage
CT = 2                 # channel slots per partition (c = 2p + j)
NG = 32                # groups (one per 4 partitions)
GS = 8                 # channels per group
EPS = 1e-5
SM_SCALE = 1.0 / 16.0  # C ** -0.5
F16 = mybir.dt.float16
F32 = mybir.dt.float32
F32R = mybir.dt.float32r
BF16 = mybir.dt.bfloat16
INT8 = mybir.dt.int8
AF = mybir.ActivationFunctionType
ALU = mybir.AluOpType

# flat int8 output: per-batch o data [C, NSP] int8, then all scales f32
# (bitcast) laid out [p, b, j] so the scale DMA is contiguous per partition
OUT_DATA = BLC * C * NSP
OUT_LEN = OUT_DATA + 128 * BLC * CT * 4
# flat int8 x input with the same tail layout for its per-channel scales
XIN_DATA = BLC * C * NSP
XIN_LEN = XIN_DATA + 128 * BLC * CT * 4

# packed const columns: gA | gnsc | gnbi | gam | bop | c2 | ident
PK_GA, PK_SC, PK_BI, PK_GAM, PK_BOP, PK_C2, PK_ID = 0, 32, 34, 36, 38, 40, 41
PK_W = 41 + 128
# wpack column layout: wmT (2*256) | wvpT (2*258) | cpack (PK_W)
WP_MT, WP_VP, WP_CP = 0, 512, 1028
WP_W = WP_CP + PK_W

_CACHE: dict = {}


def _build():
    nc = bacc.Bacc(None, target_bir_lowering=False)

    x_d = nc.dram_tensor("xin", [XIN_LEN], INT8, kind="ExternalInput")
    wpack_d = nc.dram_tensor("wpack", [128, WP_W], F32, kind="ExternalInput")
    out_d = nc.dram_tensor("out", [OUT_LEN], INT8, kind="ExternalOutput")

    with tile.TileContext(nc) as tc:
        with tc.tile_pool(name="consts", bufs=1) as consts, \
             tc.tile_pool(name="xp", bufs=4) as xp, \
             tc.tile_pool(name="xfp", bufs=4) as xfp, \
             tc.tile_pool(name="hp", bufs=2) as hp, \
             tc.tile_pool(name="gp", bufs=2) as gp, \
             tc.tile_pool(name="vp", bufs=12) as vp, \
             tc.tile_pool(name="ep", bufs=16) as ep, \
             tc.tile_pool(name="op", bufs=8) as op, \
             tc.tile_pool(name="oqp", bufs=2) as oqp, \
             tc.tile_pool(name="outp", bufs=2) as outp, \
             tc.tile_pool(name="small", bufs=6) as small, \
             tc.tile_pool(name="r2p", bufs=12) as r2p, \
             tc.tile_pool(name="cscbp", bufs=8) as cscbp, \
             tc.tile_pool(name="ps1", bufs=4, space="PSUM") as ps1, \
             tc.tile_pool(name="ps2", bufs=2, space="PSUM") as ps2:

            # ---- x scales + packed consts first (tiny), then x[0],
            #      weights, x[1..3] ----
            xsc_all = consts.tile([128, BLC, CT], F32, tag="xsc")
            nc.sync.dma_start(
                out=xsc_all,
                in_=x_d[XIN_DATA:].bitcast(F32).rearrange(
                    "(p b j) -> p b j", b=BLC, j=CT))
            cpack = consts.tile([128, PK_W], F32, tag="cpack")
            nc.sync.dma_start(out=cpack, in_=wpack_d[:, WP_CP:WP_CP + PK_W])
            gA = cpack[:, PK_GA:PK_GA + 32]
            gnsc = cpack[:, PK_SC:PK_SC + 2]
            gnbi = cpack[:, PK_BI:PK_BI + 2]
            gam = cpack[:, PK_GAM:PK_GAM + 2]
            bop = cpack[:, PK_BOP:PK_BOP + 2]
            c2t = cpack[:, PK_C2:PK_C2 + 1]
            ident = cpack[:, PK_ID:PK_ID + 128]

            def x_view(b):
                return x_d[b * C * NSP:(b + 1) * C * NSP].rearrange(
                    "(p j n) -> p j n", j=CT, n=NSP)

            xq_tiles = []
            x_sb = xp.tile([128, CT, NSP], INT8, tag="x")
            x0_src = x_view(0)
            nc.sync.dma_start(out=x_sb[:, 0, 0:512], in_=x0_src[:, 0, 0:512])
            nc.sync.dma_start(out=x_sb[:, 0, 512:1024], in_=x0_src[:, 0, 512:1024])
            nc.sync.dma_start(out=x_sb[:, 1, 0:512], in_=x0_src[:, 1, 0:512])
            nc.sync.dma_start(out=x_sb[:, 1, 512:1024], in_=x0_src[:, 1, 512:1024])
            xq_tiles.append(x_sb)
            wmT = consts.tile([128, CT, C], F32R, tag="wmT")
            nc.sync.dma_start(
                out=wmT,
                in_=wpack_d[:, WP_MT:WP_MT + 512]
                .rearrange("p (j o) -> p j o", j=CT).bitcast(F32R))
            wvpT = consts.tile([128, CT, 258], F32R, tag="wvpT")
            nc.sync.dma_start(
                out=wvpT,
                in_=wpack_d[:, WP_VP:WP_VP + 516]
                .rearrange("p (j o) -> p j o", j=CT).bitcast(F32R))
            for b in range(1, BLC):
                x_sb = xp.tile([128, CT, NSP], INT8, tag="x")
                nc.sync.dma_start(out=x_sb, in_=x_view(b))
                xq_tiles.append(x_sb)

            # dequantized fp16 x tiles (int8 * per-channel scale)
            x_tiles = [None] * BLC

            def dequant(b):
                xf = xfp.tile([128, CT, NSP], F16, tag="xf")
                for j in range(CT):
                    nc.vector.tensor_scalar_mul(
                        xf[:, j, :], xq_tiles[b][:, j, :],
                        xsc_all[:, b, j:j + 1])
                x_tiles[b] = xf

            dequant(0)

            ones = consts.tile([128, 2], F32, tag="ones")
            nc.vector.memset(ones, 1.0)
            eps_sb = consts.tile([128, 1], F32, tag="eps")
            nc.vector.memset(eps_sb, EPS)
            zeros = consts.tile([128, 1], F32, tag="zeros")
            nc.vector.memset(zeros, 0.0)

            # hoist the (single) ACT table load off the critical path
            warm = consts.tile([1, 1], F32, tag="warm")
            nc.scalar.activation(out=warm, in_=eps_sb[:1], func=AF.Exp)

            ident_bf = consts.tile([128, 128], BF16, tag="identbf")
            nc.vector.tensor_copy(out=ident_bf, in_=ident)

            # ---- GroupNorm stats: batch 0 solo (critical path), then
            #      batches 1..3 in one batched chain ----
            cs_all = cscbp.tile([128, CT, BLC], F32, tag="csall")
            cbn_all = cscbp.tile([128, CT, BLC], F32, tag="cbnall")

            def gn_stats_chain(bs):
                """bn stats -> group reduce via GpSimd partition_all_reduce
                (mask-spread trick; no PE involvement) -> rstd via ln/exp ->
                per-channel (cs, cbn)."""
                nb_ = len(bs)
                msum = small.tile([128, 2 * BLC], F32, tag="msum")
                for i, b in enumerate(bs):
                    x_sb = x_tiles[b]
                    mvs = []
                    for j in range(CT):
                        st = small.tile([128, 2, 6], F32, tag="bnst")
                        nc.vector.bn_stats(out=st[:, 0, :], in_=x_sb[:, j, 0:512])
                        nc.vector.bn_stats(out=st[:, 1, :], in_=x_sb[:, j, 512:1024])
                        mv = small.tile([128, 2], F32, tag="mv")
                        nc.vector.bn_aggr(out=mv, in_=st)
                        mvs.append(mv)
                    m2 = small.tile([128, 2], F32, tag="m2")
                    for j in range(CT):
                        nc.vector.tensor_mul(m2[:, j:j + 1], mvs[j][:, 0:1], mvs[j][:, 0:1])
                        nc.vector.tensor_add(m2[:, j:j + 1], m2[:, j:j + 1], mvs[j][:, 1:2])
                    nc.vector.tensor_add(msum[:, i:i + 1], mvs[0][:, 0:1], mvs[1][:, 0:1])
                    nc.vector.tensor_add(msum[:, nb_ + i:nb_ + i + 1], m2[:, 0:1], m2[:, 1:2])
                # spread each stat down its group's indicator column, all-reduce
                # over partitions on GpSimd, then select own group via the mask
                spread = small.tile([128, 2 * BLC, 32], F32, tag="spread")
                for i in range(2 * nb_):
                    nc.vector.tensor_scalar_mul(spread[:, i, :], gA, msum[:, i:i + 1])
                ar = small.tile([128, 2 * BLC, 32], F32, tag="ar")
                nc.gpsimd.partition_all_reduce(
                    ar[:, :2 * nb_, :], spread[:, :2 * nb_, :],
                    channels=128, reduce_op=bass_isa.ReduceOp.add)
                gsel = small.tile([128, 2 * BLC, 32], F32, tag="gsel")
                for i in range(2 * nb_):
                    nc.vector.tensor_mul(gsel[:, i, :], ar[:, i, :], gA)
                gstat = small.tile([128, 2 * BLC], F32, tag="gstat")
                nc.vector.reduce_sum(out=gstat[:, :2 * nb_], in_=gsel[:, :2 * nb_, :],
                                     axis=mybir.AxisListType.X)
                nc.vector.tensor_scalar_mul(gstat[:, :2 * nb_], gstat[:, :2 * nb_],
                                            1.0 / GS)
                gvar = small.tile([128, BLC], F32, tag="gvar")
                nc.vector.tensor_mul(gvar[:, :nb_], gstat[:, 0:nb_], gstat[:, 0:nb_])
                nc.vector.tensor_tensor(gvar[:, :nb_], gstat[:, nb_:2 * nb_],
                                        gvar[:, :nb_], ALU.subtract)
                # rstd = exp(-0.5*ln(var+eps)): keeps ACT on one table set
                nc.scalar.activation(out=gvar[:, :nb_], in_=gvar[:, :nb_],
                                     func=AF.Ln, bias=eps_sb)
                nc.scalar.activation(out=gstat[:, nb_:2 * nb_], in_=gvar[:, :nb_],
                                     func=AF.Exp, scale=-0.5)
                # per (j): cs = rstd*gnsc_j ; cbn = mean*cs - gnbi_j
                for j in range(CT):
                    for i, b in enumerate(bs):
                        nc.vector.tensor_scalar_mul(
                            cs_all[:, j, b:b + 1], gstat[:, nb_ + i:nb_ + i + 1],
                            gnsc[:, j:j + 1])
                        nc.vector.tensor_mul(cbn_all[:, j, b:b + 1],
                                             gstat[:, i:i + 1], cs_all[:, j, b:b + 1])
                        nc.vector.tensor_tensor(
                            cbn_all[:, j, b:b + 1], cbn_all[:, j, b:b + 1],
                            gnbi[:, j:j + 1], ALU.subtract)

            gn_stats_chain([0])

            # f32 scales, laid out [p, b, j] for one contiguous tail DMA
            scales_all = consts.tile([128, BLC, CT], F32, tag="scales")

            # ---- per-batch attention pipeline ----
            for b in range(BLC):
                x_sb = x_tiles[b]
                h_sb = hp.tile([128, CT, NSP], F32R, tag="h")
                for j in range(CT):
                    nc.vector.tensor_scalar(
                        out=h_sb[:, j, :], in0=x_sb[:, j, :],
                        scalar1=cs_all[:, j, b:b + 1], scalar2=cbn_all[:, j, b:b + 1],
                        op0=ALU.mult, op1=ALU.subtract,
                    )

                # ---- g = M^T h + gam: wmT cols are packed [ot][q] so the
                #      lhsT slice for output slot ot is contiguous ----
                g_sb = gp.tile([128, CT, NSP], F32R, tag="g")
                for ot in range(CT):
                    gpp = ps2.tile([128, 1024], F32, tag="ps2")
                    for nch in range(2):
                        for ct in range(CT):
                            nc.tensor.matmul(
                                gpp[:, nch * 512:(nch + 1) * 512],
                                wmT[:, ct, ot * 128:(ot + 1) * 128],
                                h_sb[:, ct, nch * 512:(nch + 1) * 512],
                                start=(ct == 0), stop=(ct == CT - 1),
                            )
                    nc.scalar.activation(out=g_sb[:, ot, :], in_=gpp,
                                         func=AF.Identity, bias=gam[:, ot:ot + 1])

                if b == 0 and BLC > 1:
                    # later batches' group stats: emitted here so their bn DMA
                    # waits sit behind batch-0's DVE work, not ahead of it
                    for bb in range(1, BLC):
                        dequant(bb)
                    gn_stats_chain(list(range(1, BLC)))

                # ---- v' (transposed, bf16) + r2t from the extra column ----
                vt = []
                r2t = []
                for mt in range(8):
                    v_t = vp.tile([128, 258], BF16, tag="vt")
                    vpp = ps1.tile([128, 512], F32, tag="ps1")
                    for ct in range(CT):
                        nc.tensor.matmul(
                            vpp[:, :258],
                            h_sb[:, ct, mt * 128:(mt + 1) * 128],
                            wvpT[:, ct, :],
                            start=(ct == 0), stop=(ct == CT - 1),
                        )
                    if mt % 2 == 0:
                        nc.scalar.activation(out=v_t[:, :256], in_=vpp[:, :256],
                                             func=AF.Copy)
                    else:
                        nc.vector.tensor_copy(out=v_t[:, :256], in_=vpp[:, :256])
                    r2 = r2p.tile([128, 1], F32, tag="r2")
                    nc.vector.tensor_tensor(r2, vpp[:, 256:257], c2t, ALU.add)
                    nc.vector.tensor_copy(out=v_t[:, 256:258], in_=ones)
                    vt.append(v_t)
                    r2t.append(r2)

                # ---- scores (transposed) + exp:
                #      E[m, n] = exp((g_m . h_n)/16 + r2t[m]) in bf16 ----
                # contraction runs over g's output channels: g slot ct holds
                # co = 2q + ct, matching h slot ct channels 2p + ct... the
                # contraction must pair g[c, m] with h[c, n] over the SAME c:
                # both operands' slot-ct tiles hold channels {2i + ct}.
                et = []
                for mt in range(8):
                    e_t = ep.tile([128, NSP], BF16, tag="et")
                    spp = ps2.tile([128, 1024], F32, tag="ps2")
                    for nch in range(2):
                        for ct in range(CT):
                            nc.tensor.matmul(
                                spp[:, nch * 512:(nch + 1) * 512],
                                g_sb[:, ct, mt * 128:(mt + 1) * 128],
                                h_sb[:, ct, nch * 512:(nch + 1) * 512],
                                start=(ct == 0), stop=(ct == CT - 1),
                            )
                    nc.scalar.activation(out=e_t, in_=spp, func=AF.Exp,
                                         scale=SM_SCALE, bias=r2t[mt])
                    et.append(e_t)

                # ---- U[n, :258] = sum_m E[m, nblock] v't[m]; normalize. ----
                ot_tiles = []
                ofull = outp.tile([128, CT, NSP], BF16, tag="osb")

                def epilogue(nb, o_t):
                    # PE-transpose oT back to channel-major, collect in bf16
                    for j in range(CT):
                        tp = ps1.tile([128, 512], BF16, tag="ps1")
                        nc.tensor.transpose(
                            tp[:, :128],
                            o_t[:, j * 128:(j + 1) * 128],
                            ident_bf,
                        )
                        nc.vector.tensor_copy(
                            out=ofull[:, j, nb * 128:(nb + 1) * 128],
                            in_=tp[:, :128])

                for nb in range(8):
                    up = ps1.tile([128, 512], F32, tag="ps1")
                    for mt in range(8):
                        nc.tensor.matmul(
                            up[:, :258],
                            et[mt][:, nb * 128:(nb + 1) * 128],
                            vt[mt],
                            start=(mt == 0), stop=(mt == 7),
                        )
                    rec = small.tile([128, 1], F32, tag="rec")
                    nc.vector.reciprocal(out=rec, in_=up[:, 256:257])
                    o_t = op.tile([128, 256], BF16, tag="ot")
                    if nb % 2 == 0:
                        nc.vector.tensor_scalar_mul(o_t, up[:, :256], rec)
                    else:
                        nc.scalar.activation(out=o_t, in_=up[:, :256],
                                             func=AF.Identity, scale=rec,
                                             bias=zeros)
                    ot_tiles.append(o_t)

                for nb in range(8):
                    epilogue(nb, ot_tiles[nb])

                # ---- per-channel int8 quant of (o + bop): the folded output
                #      bias rides inside the quantized payload, so the host
                #      dequant is just out = x + oq*scale (one fewer pass on
                #      the single host CPU that also drives the tunnel).
                #      am = absmax(o+bop), rsc = 127/am, oq = (o+bop) * rsc;
                #      scale = 1/rsc (exact inverse, bias-free dequant) ----
                for j in range(CT):
                    nc.vector.tensor_scalar_add(
                        out=ofull[:, j, :], in0=ofull[:, j, :],
                        scalar1=bop[:, j:j + 1])
                am = small.tile([128, CT], F32, tag="am")
                for j in range(CT):
                    nc.vector.tensor_reduce(
                        out=am[:, j:j + 1], in_=ofull[:, j, :],
                        op=ALU.max, axis=mybir.AxisListType.X,
                        apply_absolute_value=True)
                rsc = small.tile([128, CT], F32, tag="rsc")
                nc.vector.tensor_scalar_mul(rsc, am, 1.0 / 127.0)
                for j in range(CT):
                    nc.vector.tensor_tensor(rsc[:, j:j + 1], rsc[:, j:j + 1],
                                            eps_sb, ALU.add)
                nc.vector.reciprocal(out=rsc, in_=rsc)
                nc.vector.reciprocal(out=scales_all[:, b, :], in_=rsc)

                oq = oqp.tile([128, CT, NSP], INT8, tag="oq")
                for j in range(CT):
                    nc.vector.tensor_scalar_mul(
                        oq[:, j, :], ofull[:, j, :], rsc[:, j:j + 1])

                out_dst = out_d[b * C * NSP:(b + 1) * C * NSP].rearrange(
                    "(p j n) -> p j n", j=CT, n=NSP)
                nc.sync.dma_start(out=out_dst[:, 0, :], in_=oq[:, 0, :])
                nc.sync.dma_start(out=out_dst[:, 1, :], in_=oq[:, 1, :])

            sdst = out_d[OUT_DATA:].bitcast(F32).rearrange(
                "(p b j) -> p b j", b=BLC, j=CT)
            nc.sync.dma_start(out=sdst, in_=scales_all)

    nc.compile()
    return nc


def _col_pack(a):
    """Permute columns of [R, 256] so cols become [j][q] with co = 2q + j."""
    return a.reshape(a.shape[0], 128, 2).transpose(0, 2, 1).reshape(a.shape[0], 256)


def _prep_w(inputs):
    """Fold weights/biases (fp64) into the packed [128, WP_W] f32 tensor."""
    f64 = np.float64
    wq = np.asarray(inputs["wq"], f64)
    wk = np.asarray(inputs["wk"], f64)
    wv = np.asarray(inputs["wv"], f64)
    wo = np.asarray(inputs["wo"], f64)
    bq = np.asarray(inputs["bq"], f64)
    bk = np.asarray(inputs["bk"], f64)
    bv = np.asarray(inputs["bv"], f64)
    bo = np.asarray(inputs["bo"], f64)

    # wvpT: [C, 258]: cols 0:256 = (wo wv)^T col-packed, col 256 = (wk^T bq)/16
    wvpT = np.zeros((C, 258), np.float64)
    wvpT[:, :256] = _col_pack((wo @ wv).T)
    wvpT[:, 256] = (wk.T @ bq) * SM_SCALE

    pack = np.zeros((128, PK_W), np.float32)
    pack[np.arange(128), PK_GA + np.arange(128) // 4] = 1.0      # gA
    pack[:, PK_SC:PK_SC + 2] = np.asarray(inputs["gn_scale"], np.float32).reshape(128, 2)
    pack[:, PK_BI:PK_BI + 2] = np.asarray(inputs["gn_bias"], np.float32).reshape(128, 2)
    pack[:, PK_GAM:PK_GAM + 2] = (wq.T @ bk).astype(np.float32).reshape(128, 2)
    pack[:, PK_BOP:PK_BOP + 2] = (wo @ bv + bo).astype(np.float32).reshape(128, 2)
    pack[:, PK_C2] = np.float32(float(bq @ bk) * SM_SCALE)
    pack[:, PK_ID:PK_ID + 128] = np.eye(128, dtype=np.float32)

    wmT = _col_pack(wk.T @ wq).astype(np.float32)                # [C, C], c = 2p+j

    wpk = np.empty((128, WP_W), np.float32)
    wpk[:, WP_MT:WP_MT + 512] = wmT.reshape(128, 2, 256).reshape(128, 512)
    wpk[:, WP_VP:WP_VP + 516] = wvpT.astype(np.float32).reshape(128, 2, 258).reshape(128, 516)
    wpk[:, WP_CP:WP_CP + PK_W] = pack
    return wpk


def _get_state():
    if "st" in _CACHE:
        return _CACHE["st"]
    import jax
    from jax.sharding import Mesh, PartitionSpec, NamedSharding
    from jax.experimental.shard_map import shard_map
    from concourse import bass2jax

    bass2jax.install_neuronx_cc_hook()
    nc = _build()
    _CACHE["nc"] = nc

    partition_name = nc.partition_id_tensor.name if nc.partition_id_tensor else None
    in_names, out_names, out_avals = [], [], []
    for alloc in nc.m.functions[0].allocations:
        if not isinstance(alloc, mybir.MemoryLocationSet):
            continue
        name = alloc.memorylocations[0].name
        if alloc.kind == "ExternalInput":
            if name != partition_name:
                in_names.append(name)
        elif alloc.kind == "ExternalOutput":
            assert alloc.tensor_shape is not None and alloc.dtype is not None
            out_names.append(name)
            out_avals.append(
                jax.core.ShapedArray(tuple(alloc.tensor_shape), mybir.dt.np(alloc.dtype)))
    assert in_names == ["xin", "wpack"], in_names
    assert out_names == ["out"], out_names
    assert nc.dbg_addr is None

    devices = jax.devices()[:N_CORES]
    mesh = Mesh(np.asarray(devices), ("core",))
    sh = NamedSharding(mesh, PartitionSpec("core"))

    def _body(x, w):
        operands = [x, w]
        if partition_name is not None:
            operands.append(bass2jax.partition_id_tensor())
        outs = bass2jax._bass_exec_p.bind(
            *operands,
            out_avals=tuple(out_avals),
            in_names=tuple(in_names + ([partition_name] if partition_name else [])),
            out_names=tuple(out_names),
            lowering_input_output_aliases=(),
            sim_require_finite=True,
            sim_require_nnan=True,
            nc=nc,
        )
        return outs[0]

    sharded = jax.jit(
        shard_map(
            _body, mesh=mesh,
            in_specs=(PartitionSpec("core"), PartitionSpec("core")),
            out_specs=PartitionSpec("core"), check_rep=False,
        ),
        keep_unused=True,
    )

    st = {"sharded": sharded, "sh": sh, "wcache": {}}
    _CACHE["st"] = st
    return st


def _finish_stage(buf, x, out, k):
    """out[stage k] = x + oq*scale (bop is folded into oq on device).

    Runs inline — the host has a single CPU that also drives the tunnel
    streams, so fewer passes beat more threads.
    """
    buf = buf.reshape(N_CORES, OUT_LEN)
    for core in range(N_CORES):
        blk = buf[core]
        oq = blk[:OUT_DATA].reshape(BLC, C, NSP)
        sc = blk[OUT_DATA:].view(np.float32).reshape(128, BLC, CT)
        scb = np.ascontiguousarray(sc.transpose(1, 0, 2)).reshape(BLC, C)  # c = 2p+j
        for bl in range(BLC):
            gb = core * BL + k * BLC + bl
            o = out[gb]
            np.multiply(oq[bl], scb[bl][:, None], out=o)
            o += x[gb]


def _to_int8_stage(x, k):
    """Per-(b,c) int8 quant of stage k's batches -> per-core flat [8*XIN_LEN]."""
    xq = np.empty((N_CORES, XIN_LEN), np.int8)
    for core in range(N_CORES):
        b0 = core * BL + k * BLC
        blk = x[b0:b0 + BLC]                        # [BLC, C, NSP]
        am = np.maximum(blk.max(axis=2), -blk.min(axis=2))  # [BLC, C]
        np.maximum(am, 1e-12, out=am)
        # offset-binary round: uint8(v + 128.5) ^ 0x80 == int8(round(v))
        q = (blk * (127.0 / am)[:, :, None] + 128.5).astype(np.uint8)
        q ^= 0x80
        xq[core, :XIN_DATA] = q.reshape(-1).view(np.int8)
        sc = (am * (1.0 / 127.0)).astype(np.float32)
        scp = sc.reshape(BLC, 128, CT).transpose(1, 0, 2)  # [p, b, j], c = 2p+j
        xq[core, XIN_DATA:].view(np.float32)[:] = scp.reshape(-1)
    return xq.reshape(-1)


def _run(inputs):
    import jax
    import threading

    st = _get_state()
    x = np.asarray(inputs["x"], np.float32).reshape(B, C, NSP)

    import hashlib
    hsh = hashlib.blake2b(digest_size=16)
    for name in ("gn_scale", "gn_bias", "wq", "bq", "wk", "bk",
                 "wv", "bv", "wo", "bo"):
        hsh.update(np.ascontiguousarray(inputs[name]).tobytes())
    wkey = hsh.digest()
    if st.get("wprep_key") == wkey:
        wpk = st["wprep"]
    else:
        wpk = _prep_w(inputs)
        st["wprep_key"], st["wprep"] = wkey, wpk
    out = np.empty((B, C, NSP), np.float32)

    def _exec():
        # pipeline: stage k+1's quant+upload overlaps stage k's pull
        threads, errs = [], []

        def pull_and_finish(r, k):
            try:
                _finish_stage(np.asarray(r), x, out, k)
            except Exception as e:  # re-raised after joins
                errs.append(e)

        for k in range(PIPE):
            xq = _to_int8_stage(x, k)
            if st["wcache"].get(wkey) is None:
                wg = np.broadcast_to(wpk, (N_CORES, 128, WP_W)).reshape(
                    N_CORES * 128, WP_W)
                xd, wdev = jax.device_put((xq, np.ascontiguousarray(wg)),
                                          (st["sh"], st["sh"]))
                st["wcache"].clear()
                st["wcache"][wkey] = wdev
            else:
                xd = jax.device_put(xq, st["sh"])
                wdev = st["wcache"][wkey]
            r = st["sharded"](xd, wdev)
            try:
                r.copy_to_host_async()
            except Exception:
                pass
            th = threading.Thread(target=pull_and_finish, args=(r, k))
            th.start()
            threads.append(th)
        for th in threads:
            th.join()
        if errs:
            raise errs[0]

    try:
        _exec()
    except Exception as e:
        if "LoadExecutable" not in str(e):
            raise
        # A jax persistent-cache executable can go stale when the axon
        # terminal restarts; recompile with the cache off and retry once.
        jax.config.update("jax_enable_compilation_cache", False)
        jax.clear_caches()
        st["wcache"].clear()
        _exec()

    return out.reshape(B, C, H, W), None


def kernel(**inputs) -> np.ndarray:
    out, _ = _run(inputs)
    return out
